# revision 9
# baseline (speedup 1.0000x reference)
"""Difference 3D cost volume on 8 Trainium2 NeuronCores.

cost[n,c,d,h,w] = l[n,c,h,w] - r[n,c,h,w-d]  (w >= d), else 1.0
Shapes: l,r [2,32,128,256] f32 -> out [2,32,48,128,256] f32.

Sharding: data-parallel over the 64 (n,c) slices, 8 per core. Each core
computes, per slice, the full [H, D, W] volume in CH-disparity chunks:
one fused tensor_sub per chunk (broadcast l over d via stride-0 AP,
shift r via stride -1 AP into a 48-col left-padded copy) and one
contiguous multi-MB store in [h, d, w] order. Every OFFLOAD-th chunk's
subtract runs on GpSimd, which never contends with DVE's fp32
tensor_tensor (1-port mode), adding compute throughput. Host gather
transposes [h,d] -> [d,h] and writes the constant-1.0 prefixes (w < d),
which the device leaves as garbage.
"""

import numpy as np

N, C, H, W, D = 2, 32, 128, 256, 48
PAD = 48  # left pad on r rows; must be >= D
NCORES = 8
PAIRS = N * C
PPC = PAIRS // NCORES  # (n,c) slices per core
CH = 8  # disparities per compute/store chunk (divides D)
OFFLOAD = 3  # if >0, every OFFLOAD-th chunk's subtract runs on GpSimd

_nc_cache = None
_runner_cache = None


def _emit(tc, lf, rf, out):
    """Emit the per-core program. lf [PPC,H,W], rf [PPC,H,PAD+W],
    out [PPC,H,D,W] viewed as [PPC,H,D*W]."""
    from concourse import mybir
    from contextlib import ExitStack

    nc = tc.nc
    ov = out.rearrange("p h d w -> p h (d w)")
    with ExitStack() as ctx:
        lp = ctx.enter_context(tc.tile_pool(name="lp", bufs=4))
        rp = ctx.enter_context(tc.tile_pool(name="rp", bufs=4))
        op = ctx.enter_context(tc.tile_pool(name="op", bufs=6))
        for p in range(PPC):
            lt = lp.tile([H, W], mybir.dt.float32)
            nc.scalar.dma_start(lt[:], lf[p])
            rt = rp.tile([H, PAD + W], mybir.dt.float32)
            nc.scalar.dma_start(rt[:], rf[p])

            for c in range(D // CH):
                d0 = c * CH
                ot = op.tile([H, CH * W], mybir.dt.float32)

                # out[h, d*W + w] = l[h, w] - rpad[h, PAD - d + w]
                l_ap = lt[:, 0:W]
                l_ap.ap = l_ap.ap[:-1] + [[0, CH], [1, W]]
                r_ap = rt[:, PAD - d0 : PAD - d0 + W]
                r_ap.ap = r_ap.ap[:-1] + [[-1, CH], [1, W]]
                o_ap = ot[:, 0 : CH * W]
                o_ap.ap = o_ap.ap[:-1] + [[W, CH], [1, W]]
                eng = (
                    nc.gpsimd
                    if OFFLOAD and c % OFFLOAD == OFFLOAD - 1
                    else nc.vector
                )
                eng.tensor_sub(o_ap, l_ap, r_ap)

                nc.sync.dma_start(ov[p][:, d0 * W : (d0 + CH) * W], ot[:])


def _build():
    global _nc_cache
    if _nc_cache is not None:
        return _nc_cache
    import concourse.tile as tile
    from concourse import bacc, mybir

    nc = bacc.Bacc(
        "TRN2", target_bir_lowering=False, debug=False, num_devices=NCORES
    )
    lf = nc.dram_tensor("lf", [PPC, H, W], mybir.dt.float32, kind="ExternalInput").ap()
    rf = nc.dram_tensor(
        "rf", [PPC, H, PAD + W], mybir.dt.float32, kind="ExternalInput"
    ).ap()
    out = nc.dram_tensor(
        "out", [PPC, H, D, W], mybir.dt.float32, kind="ExternalOutput"
    ).ap()
    with tile.TileContext(nc) as tc:
        _emit(tc, lf, rf, out)
    nc.compile()
    _nc_cache = nc
    return nc


def _get_runner():
    """Build (once) a cached PJRT executable over the 8-core mesh.

    No donation: the zero output-operands stay resident on device and are
    reused every call; the NEFF writes every output byte so uninitialized
    result buffers are fine.
    """
    global _runner_cache
    if _runner_cache is not None:
        return _runner_cache

    import jax
    from jax.sharding import Mesh, NamedSharding, PartitionSpec

    import concourse.mybir as mybir
    from concourse.bass2jax import (
        _bass_exec_p,
        install_neuronx_cc_hook,
        partition_id_tensor,
    )

    try:
        from jax.experimental.shard_map import shard_map
    except ImportError:
        from jax.shard_map import shard_map

    nc = _build()
    install_neuronx_cc_hook()
    partition_name = nc.partition_id_tensor.name if nc.partition_id_tensor else None

    in_names, out_names, out_avals, zero_outs = [], [], [], []
    for alloc in nc.m.functions[0].allocations:
        if not isinstance(alloc, mybir.MemoryLocationSet):
            continue
        name = alloc.memorylocations[0].name
        if alloc.kind == "ExternalInput":
            if name != partition_name:
                in_names.append(name)
        elif alloc.kind == "ExternalOutput":
            shape = tuple(alloc.tensor_shape)
            dtype = mybir.dt.np(alloc.dtype)
            out_names.append(name)
            out_avals.append(jax.core.ShapedArray(shape, dtype))
            zero_outs.append(np.zeros(shape, dtype))
    all_in_names = list(in_names) + list(out_names)
    if partition_name is not None:
        all_in_names.append(partition_name)

    def _body(*args):
        operands = list(args)
        if partition_name is not None:
            operands.append(partition_id_tensor())
        outs = _bass_exec_p.bind(
            *operands,
            out_avals=tuple(out_avals),
            in_names=tuple(all_in_names),
            out_names=tuple(out_names),
            lowering_input_output_aliases=(),
            sim_require_finite=True,
            sim_require_nnan=True,
            nc=nc,
        )
        return tuple(outs)

    devices = jax.devices()[:NCORES]
    mesh = Mesh(np.asarray(devices), ("core",))
    nin = len(in_names)
    nout = len(out_names)
    fn = jax.jit(
        shard_map(
            _body,
            mesh=mesh,
            in_specs=(PartitionSpec("core"),) * (nin + nout),
            out_specs=(PartitionSpec("core"),) * nout,
            check_rep=False,
        ),
        keep_unused=True,
    )
    sharding = NamedSharding(mesh, PartitionSpec("core"))
    zeros_dev = [
        jax.device_put(
            np.zeros((NCORES * z.shape[0], *z.shape[1:]), z.dtype), sharding
        )
        for z in zero_outs
    ]
    _runner_cache = (fn, in_names, zeros_dev, sharding)
    return _runner_cache


def _prep_inputs(l_fmap, r_fmap):
    l = np.ascontiguousarray(np.asarray(l_fmap, dtype=np.float32)).reshape(
        PAIRS, H, W
    )
    r = np.ascontiguousarray(np.asarray(r_fmap, dtype=np.float32)).reshape(
        PAIRS, H, W
    )
    rpad = np.zeros((PAIRS, H, PAD + W), np.float32)
    rpad[:, :, PAD:] = r
    return {"lf": l, "rf": rpad}


def _gather(out_global):
    """[PAIRS,H,D,W] device result -> [N,C,D,H,W] with 1.0 prefixes."""
    full = np.asarray(out_global).reshape(N, C, H, D, W)
    out = np.ascontiguousarray(np.moveaxis(full, 2, 3))  # [N,C,D,H,W]
    for d in range(1, D):
        out[:, :, d, :, :d] = 1.0
    return out


def kernel(l_fmap, r_fmap):
    import jax

    fn, in_names, zeros_dev, sharding = _get_runner()
    named = _prep_inputs(l_fmap, r_fmap)
    concat_in = [jax.device_put(named[name], sharding) for name in in_names]
    out_arrs = fn(*concat_in, *zeros_dev)
    return _gather(out_arrs[0])


def run(l_fmap, r_fmap, trace=False):
    """Legacy path via run_bass_kernel_spmd (used by test.py)."""
    from concourse.bass_utils import run_bass_kernel_spmd

    named = _prep_inputs(l_fmap, r_fmap)
    in_maps = [
        {k: np.ascontiguousarray(v[c * PPC : (c + 1) * PPC]) for k, v in named.items()}
        for c in range(NCORES)
    ]
    nc = _build()
    res = run_bass_kernel_spmd(
        nc, in_maps, core_ids=list(range(NCORES)), trace=trace
    )
    parts = [res.results[k]["out"] for k in range(NCORES)]
    out = _gather(np.concatenate(parts, axis=0))
    return out, res
